# revision 25
# baseline (speedup 1.0000x reference)
"""TRN2 Bass kernel for nn_BioSSMMixer.

Sharding: 8 cores = DP over batch (2) x TP over D-channels (4 x 672).
Per core: bf16 cat-GEMM over 18 x 128-wide padded blocks [W_in|W_z|W_dt]
(128-col weights keep FWL) plus a stats/BC block [ones|W_B|W_C] computed a
chunk ahead; the LayerNorm mean-correction is folded into the contraction
as one extra matmul per block against a broadcast mu (lhsT = outer(ones,
-colsum/128)), so the PSUM epilogue is a single rB-scale mul per block;
fp32 tensor_tensor_scan for the SSM state (bf16 forcing); nonlinear
spiking membrane scan runs chunk-parallel over time (32 chunks of 64
steps + 64 warmup steps); ReduceScatter of the partial out-GEMM within
each 4-core group; final token-sharded output.
"""
import sys, types

sys.path.insert(0, "/opt/trn_rl_repo")

# Inject the missing antenv.axon_hooks so trace=True can profile via NTFF.
try:
    import antenv

    if "antenv.axon_hooks" not in sys.modules:
        _m = types.ModuleType("antenv.axon_hooks")
        _m._hook = None

        def _set(h):
            _m._hook = h

        def _get():
            return _m._hook

        _m.set_axon_ntff_profile_hook = _set
        _m.get_axon_ntff_profile_hook = _get
        sys.modules["antenv.axon_hooks"] = _m
        antenv.axon_hooks = _m
        try:
            from trn_agent_boot.trn_boot import _ntff_profile_via_ctypes

            hk = _ntff_profile_via_ctypes("/opt/axon/libaxon_pjrt.so")
            if hk is not None:
                _m._hook = hk
        except Exception:
            pass
except Exception:
    pass

import numpy as np
import ml_dtypes

import concourse.bass as bass
import concourse.mybir as mybir
import concourse.tile as tile
from concourse import bacc
from concourse.bass_utils import run_bass_kernel_spmd

F32 = mybir.dt.float32
BF16 = mybir.dt.bfloat16
AF = mybir.ActivationFunctionType
OP = mybir.AluOpType

# ---- problem constants (hardcoded per the harness contract) ----
D, T, B, N, KG = 2688, 2048, 2, 4, 16
V_TH_MIN, SPIKE_BETA, V_DECAY, LN_EPS = 0.1, 4.0, 0.9, 1e-5
NCORE = 8
QD = D // 4            # 672 channels per core
P112 = 112             # partition rows per g-group
G6 = 6                 # g-groups per core (112*6 = 672)
TC = 512               # time chunk for GEMM/scan phases
NTC = T // TC          # 4
KT = D // 128          # 21 k-tiles
NB = 18                # main cat blocks: (112 real + 16 pad) each
CATW = (NB + 1) * 128  # + stats/BC block [ones|W_B|W_C|pad]
NCHUNK = 64            # membrane scan chunks
LCH = T // NCHUNK      # 64
WARM = 32              # membrane warmup steps = LCH (0.9^32; sim-validated)
NOUT_CH = 448          # out-GEMM N chunk (6*448 = 2688)

bf16r = lambda x: np.ascontiguousarray(np.asarray(x, np.float32).astype(ml_dtypes.bfloat16))

_CACHE = {}


def _build():
    nc = bacc.Bacc("TRN2", target_bir_lowering=False, debug=False, num_devices=NCORE)

    hT = nc.declare_dram_parameter("hT", [NTC, KT, 128, TC], BF16, isOutput=False)
    wcat = nc.declare_dram_parameter("wcat", [NB, 128, KT * 128], BF16, isOutput=False)
    w18 = nc.declare_dram_parameter("w18", [128, KT * 128], BF16, isOutput=False)
    csmat = nc.declare_dram_parameter("csmat", [128, NB * 128], BF16, isOutput=False)
    csb9 = nc.declare_dram_parameter("csb9", [9, 1], F32, isOutput=False)
    selmu = nc.declare_dram_parameter("selmu", [9, 128], BF16, isOutput=False)
    sel9 = nc.declare_dram_parameter("sel9", [9, 8 * P112], BF16, isOutput=False)
    wout = nc.declare_dram_parameter("wout", [G6, P112, D], BF16, isOutput=False)
    aperm = nc.declare_dram_parameter("aperm", [P112, G6 * N], F32, isOutput=False)
    vb4 = nc.declare_dram_parameter("vb4", [P112, 1], F32, isOutput=False)
    vbn = nc.declare_dram_parameter("vbn", [P112, 1], F32, isOutput=False)
    bdt = nc.declare_dram_parameter("bdt", [P112, G6], F32, isOutput=False)
    onesc = nc.declare_dram_parameter("onesc", [128, 1], BF16, isOutput=False)
    ones1 = nc.declare_dram_parameter("ones1", [1, 128], BF16, isOutput=False)
    ones1f = nc.declare_dram_parameter("ones1f", [1, 128], F32, isOutput=False)
    outp = nc.declare_dram_parameter("out", [4, TC // 4, D], BF16, isOutput=True)

    with tile.TileContext(nc) as tc:
        with (
            tc.tile_pool(name="consts", bufs=1) as cpool,
            tc.tile_pool(name="ybuf", bufs=1) as ypool,
            tc.tile_pool(name="dram", bufs=1, space="DRAM") as dpool,
        ):
            # ---- load constants to SBUF ----
            a_sb = cpool.tile([P112, G6 * N], F32)
            vb4_sb = cpool.tile([P112, 1], F32)
            vbn_sb = cpool.tile([P112, 1], F32)
            bdt_sb = cpool.tile([P112, G6], F32)
            cs_sb = cpool.tile([128, NB * 128], BF16)
            csb9_sb = cpool.tile([9, 1], F32)
            selmu_sb = cpool.tile([9, 128], BF16)
            sel9_sb = cpool.tile([9, 8 * P112], BF16)
            onesc_sb = cpool.tile([128, 1], BF16)
            ones1_sb = cpool.tile([1, 128], BF16)
            ones1f_sb = cpool.tile([1, 128], F32)
            w18_sb = cpool.tile([128, KT * 128], BF16)
            for dst, src in [(a_sb, aperm), (vb4_sb, vb4), (vbn_sb, vbn),
                             (bdt_sb, bdt), (cs_sb, csmat), (csb9_sb, csb9),
                             (selmu_sb, selmu), (sel9_sb, sel9),
                             (onesc_sb, onesc), (ones1_sb, ones1),
                             (ones1f_sb, ones1f), (w18_sb, w18)]:
                nc.sync.dma_start(out=dst[:], in_=src[:])

            # persistent big buffers
            y_bf = ypool.tile([P112, G6 * T], BF16)     # y, tau-major time
            yz_bf = ypool.tile([P112, G6 * T], BF16)    # y*z, token-major
            s_carry = cpool.tile([P112, G6 * N], F32)   # scan carries

            with (
                tc.tile_pool(name="ht", bufs=1) as htp,
                tc.tile_pool(name="w", bufs=2) as wp,
                tc.tile_pool(name="sq", bufs=2) as sqp,
                tc.tile_pool(name="udt", bufs=1) as udtp,
                tc.tile_pool(name="zp", bufs=1) as zpool,
                tc.tile_pool(name="scr", bufs=1) as scr,
                tc.tile_pool(name="rows", bufs=1) as rowp,
                tc.tile_pool(name="ps_gemm", bufs=3, space="PSUM") as psg,
                tc.tile_pool(name="ps_st", bufs=1, space="PSUM") as pssq,
                tc.tile_pool(name="ps_bc", bufs=1, space="PSUM") as psbc,
            ):
                def stage_load(tci):
                    """hts, stats/BC block, LN stats, muB/rB broadcasts."""
                    hts = []
                    for k in range(KT):
                        ht_t = htp.tile([128, TC], BF16, tag=f"ht{k}", name=f"ht{k}")
                        nc.sync.dma_start(out=ht_t[:], in_=hT[tci, k])
                        hts.append(ht_t)
                    # block 18: [ones | W_B | W_C] -> sum row + raw Bm/Cm rows
                    ps18 = psbc.tile([128, TC], F32, tag="bc18", name="ps18")
                    for k in range(KT):
                        nc.tensor.matmul(ps18[:], w18_sb[:, k * 128:(k + 1) * 128],
                                         hts[k][:], start=(k == 0), stop=(k == KT - 1))
                    raw = rowp.tile([9, TC], BF16, tag=f"raw{tci % 2}", name="raw")
                    nc.vector.tensor_copy(raw[:], ps18[0:9, :])
                    # sumsq via Square + ones-column reduce
                    ps_sq = pssq.tile([1, TC], F32, tag="psq", name="psq")
                    for k in range(KT):
                        sq_t = sqp.tile([128, TC], BF16, tag=f"sq{k % 2}")
                        nc.gpsimd.tensor_mul(sq_t[:], hts[k][:], hts[k][:])
                        nc.tensor.matmul(ps_sq[:], onesc_sb[:], sq_t[:],
                                         start=(k == 0), stop=(k == KT - 1))
                    # mu broadcast to all 128 partitions (bf16)
                    mps = psbc.tile([128, TC], F32, tag="bc", bufs=2)
                    nc.tensor.matmul(mps[:], selmu_sb[:], raw[:])
                    muB = scr.tile([128, TC], BF16, tag=f"muB{tci % 2}", name="muB")
                    nc.vector.tensor_copy(muB[:], mps[:])
                    # mean-corrected raw BC rows: rawc = raw - mu*colsum
                    rawc = rowp.tile([9, TC], BF16, tag=f"rawc{tci % 2}", name="rawc")
                    nc.vector.scalar_tensor_tensor(rawc[:], muB[0:9, :],
                                                   csb9_sb[:, 0:1], raw[:],
                                                   OP.mult, OP.add)
                    m2 = rowp.tile([1, TC], F32, tag="m2")
                    nc.vector.tensor_mul(m2[:], muB[0:1, :], muB[0:1, :])
                    var = rowp.tile([1, TC], F32, tag="var")
                    nc.vector.scalar_tensor_tensor(var[:], ps_sq[:], 1.0 / D,
                                                   m2[:], OP.mult, OP.subtract)
                    vare = rowp.tile([1, TC], F32, tag="m2")
                    nc.vector.tensor_scalar_add(vare[:], var[:], LN_EPS)
                    sd = rowp.tile([1, TC], F32, tag="sd")
                    nc.scalar.activation(sd[:], vare[:], AF.Sqrt)
                    r = rowp.tile([1, TC], F32, tag="var")
                    nc.vector.reciprocal_approx_fast(out=r[:], in_=sd[:])
                    rps = psbc.tile([128, TC], F32, tag="bc", bufs=2)
                    nc.tensor.matmul(rps[:], ones1f_sb[:], r[:])
                    rB = scr.tile([128, TC], F32, tag=f"rB{tci % 2}", name="rB")
                    nc.vector.tensor_copy(rB[:], rps[:])
                    return hts, rB, muB, rawc

                yv = y_bf[:].rearrange("p (tau c g) -> p c tau g",
                                       tau=LCH, c=NCHUNK, g=G6)
                CPT = TC // LCH

                def scan_core(tci, g, u_t, dt_t, BmB, CmB):
                    # SSM state scan + y for one g-group (yz deferred until z)
                    du = scr.tile([P112, TC], BF16, tag="du")
                    nc.vector.tensor_mul(du[:], dt_t[g][:], u_t[g][:])
                    s_of_n = []
                    for n in range(N):
                        dec = scr.tile([P112, TC], F32, tag=f"dec{n}")
                        nc.scalar.activation(dec[:], dt_t[g][:], AF.Exp,
                                             scale=a_sb[:, g * N + n:g * N + n + 1])
                        inp = scr.tile([P112, TC], BF16, tag=f"inp{n % 2}")
                        nc.vector.tensor_mul(inp[:], du[:], BmB[n][:])
                        s_t = scr.tile([P112, TC], F32, tag=f"s{n}")
                        ini = 0.0 if tci == 0 else s_carry[:, g * N + n:g * N + n + 1]
                        nc.vector.tensor_tensor_scan(s_t[:], dec[:], inp[:], ini,
                                                     OP.mult, OP.add)
                        nc.vector.tensor_copy(s_carry[:, g * N + n:g * N + n + 1],
                                              s_t[:, TC - 1:TC])
                        s_of_n.append(s_t)
                    yac = scr.tile([P112, TC], F32, tag="yac")
                    tmp = scr.tile([P112, TC], F32, tag="ytmp")
                    nc.vector.tensor_mul(yac[:], s_of_n[0][:], CmB[0][:])
                    nc.vector.tensor_mul(tmp[:], s_of_n[1][:], CmB[1][:])
                    nc.gpsimd.tensor_add(yac[:], yac[:], tmp[:])
                    nc.vector.tensor_mul(tmp[:], s_of_n[2][:], CmB[2][:])
                    nc.gpsimd.tensor_add(yac[:], yac[:], tmp[:])
                    nc.vector.tensor_mul(tmp[:], s_of_n[3][:], CmB[3][:])
                    ysl = yv[:, CPT * tci:CPT * (tci + 1), :, g:g + 1]
                    nc.vector.tensor_add(ysl, yac[:], tmp[:])

                def gemm_block(jt, hts, muB, rB, dst):
                    wt = wp.tile([128, KT * 128], BF16, tag="w")
                    nc.sync.dma_start(out=wt[:], in_=wcat[jt])
                    ps = psg.tile([128, TC], F32, tag="psg")
                    for k in range(KT):
                        nc.tensor.matmul(ps[:], wt[:, k * 128:(k + 1) * 128],
                                         hts[k][:], start=(k == 0), stop=False)
                    nc.tensor.matmul(ps[:], cs_sb[:, jt * 128:(jt + 1) * 128],
                                     muB[:], start=False, stop=True)
                    nc.vector.tensor_mul(dst[:], ps[0:P112, :], rB[0:P112, :])

                staged = {0: stage_load(0)}
                for tci in range(NTC):
                    hts, rB, muB, rawc = staged.pop(tci)
                    u_t = {g: udtp.tile([P112, TC], BF16, tag=f"u{g}", name=f"u{g}") for g in range(G6)}
                    dt_t = {g: udtp.tile([P112, TC], BF16, tag=f"dt{g}", name=f"dtt{g}") for g in range(G6)}
                    zpre = {g: zpool.tile([P112, TC], BF16, tag=f"zp{g}", name=f"zpre{g}") for g in range(G6)}
                    dpre = {g: zpool.tile([P112, TC], F32, tag=f"dp{g}", name=f"dpre{g}") for g in range(G6)}
                    # Bm/Cm broadcasts first (rB-scale folded into the evac mul)
                    BmB, CmB = {}, {}
                    for n in range(2 * N):
                        b_ps = psbc.tile([P112, TC], F32, tag="bc", bufs=2)
                        nc.tensor.matmul(b_ps[:], sel9_sb[:, n * P112:(n + 1) * P112],
                                         rawc[:])
                        b_sb = scr.tile([P112, TC], BF16, tag=f"bc{n}_{tci % 2}", name=f"bc{n}")
                        nc.vector.tensor_mul(b_sb[:], b_ps[:], rB[0:P112, :])
                        (BmB if n < N else CmB)[n % N] = b_sb
                    # g-pair-batched u/dt blocks with this chunk's scans
                    # interleaved; pairing halves the ACT table reloads.
                    for gp in range(0, G6, 2):
                        e_t = {}
                        for g in (gp, gp + 1):
                            gemm_block(g, hts, muB, rB, u_t[g])
                            gemm_block(12 + g, hts, muB, rB, dpre[g])
                        for g in (gp, gp + 1):
                            e_t[g] = zpool.tile([P112, TC], BF16, tag=f"e{g % 2}", name=f"et{g}")
                            nc.scalar.activation(e_t[g][:], dpre[g][:], AF.Exp,
                                                 bias=bdt_sb[:, g:g + 1])
                        for g in (gp, gp + 1):
                            nc.scalar.activation(dt_t[g][:], e_t[g][:], AF.Ln, bias=1.0)
                        for g in (gp, gp + 1):
                            scan_core(tci, g, u_t, dt_t, BmB, CmB)
                    # z blocks + sigmoid + yz
                    for g in range(G6):
                        gemm_block(6 + g, hts, muB, rB, zpre[g])
                    z_t = {g: zpool.tile([P112, TC], BF16, tag=f"z{g}", name=f"zt{g}") for g in range(G6)}
                    for g in range(G6):
                        nc.scalar.activation(z_t[g][:], zpre[g][:], AF.Sigmoid)
                    for g in range(G6):
                        eng = nc.gpsimd if g % 2 == 0 else nc.vector
                        eng.tensor_mul(
                            yz_bf[:, g * T + tci * TC: g * T + (tci + 1) * TC],
                            yv[:, CPT * tci:CPT * (tci + 1), :, g:g + 1], z_t[g][:])

                    # prefetch next chunk's stats
                    if tci + 1 < NTC:
                        staged[tci + 1] = stage_load(tci + 1)

            # ========== membrane scan + overlapped tail ==========
            wop = tc.alloc_tile_pool(name="wo", bufs=1)
            wo = []
            for g in range(G6):
                wt = wop.tile([P112, D], BF16, tag=f"wo{g}", name=f"wo{g}")
                nc.sync.dma_start(out=wt[:], in_=wout[g])
                wo.append(wt)
            NQ = 4
            HT2 = T // NQ          # 512 tokens per quarter
            part_b = [dpool.tile([HT2, D], BF16, name=f"partb{h}") for h in range(NQ)]
            rs_out = [dpool.tile([HT2 // 4, D], BF16, name=f"rsout{h}") for h in range(NQ)]
            WAL = NCHUNK * G6          # columns per tau row
            CHAINS = [(0, 22), (22, 22), (44, 20)]   # (c_lo, n_chunks) per chain
            with (
                tc.tile_pool(name="spk", bufs=1) as spp,
                tc.tile_pool(name="vv", bufs=1) as vvp,
                tc.tile_pool(name="vpre", bufs=3) as vpp,
                tc.tile_pool(name="oev", bufs=1) as oevp,
                tc.tile_pool(name="ps_o", bufs=4, space="PSUM") as pso,
            ):
                # spike buffer shares y_bf's tau-major layout: free = tau*192 + c*6 + g
                sp_bf = spp.tile([P112, G6 * T], BF16, name="spbf")
                spv = sp_bf[:].rearrange("p (tau c g) -> p c tau g",
                                         tau=LCH, c=NCHUNK, g=G6)
                v_c, spw = {}, {}
                for x, (c_lo, ncc) in enumerate(CHAINS):
                    hcx = ncc * G6
                    v_c[x] = vvp.tile([P112, hcx], BF16, tag=f"v{x}", name=f"v{x}")
                    spw[x] = vvp.tile([P112, hcx], BF16, tag=f"sw{x}", name=f"sw{x}")
                    nc.vector.memset(v_c[x][:], 0.0)

                def vstep(tau, warm, x):
                    c_lo, ncc = CHAINS[x]
                    hcx = ncc * G6
                    if warm:
                        lo = max(c_lo, 1)          # chunk 0 has no warmup
                        vs = v_c[x][:, (lo - c_lo) * G6:hcx]
                        yo = (LCH + tau) * WAL + (lo - 1) * G6
                        wdt = (c_lo + ncc - lo) * G6
                        sps = spw[x][:, (lo - c_lo) * G6:hcx]
                    else:
                        vs = v_c[x][:, 0:hcx]
                        yo = tau * WAL + c_lo * G6
                        wdt = hcx
                        sps = sp_bf[:, tau * WAL + c_lo * G6:
                                    tau * WAL + c_lo * G6 + hcx]
                    ys = y_bf[:, yo:yo + wdt]
                    vp = vpp.tile([P112, 22 * G6], BF16, tag=f"vp{x}", name=f"vp{x}")
                    vps = vp[:, 0:wdt]
                    nc.vector.scalar_tensor_tensor(vps, vs, V_DECAY, ys, OP.mult, OP.add)
                    nc.scalar.activation(sps, vps, AF.Sigmoid,
                                         bias=vb4_sb[:, 0:1], scale=SPIKE_BETA)
                    nc.vector.scalar_tensor_tensor(vs, sps, vbn_sb[:, 0:1], vps,
                                                   OP.mult, OP.add)

                for tau in range(-WARM, 0):
                    for x in range(len(CHAINS)):
                        vstep(tau, True, x)
                for tau in range(LCH):
                    for x in range(len(CHAINS)):
                        vstep(tau, False, x)

                CPT = TC // LCH
                NCH = D // NOUT_CH
                TTQ = HT2 // 128

                for h in range(NQ):
                    # g = spike * (y*z) for this quarter only (keep gpsimd
                    # clear of the collective queue)
                    for g in range(G6):
                        sl = slice(g * T + h * TC, g * T + (h + 1) * TC)
                        nc.vector.tensor_mul(yz_bf[:, sl],
                                             spv[:, CPT * h:CPT * (h + 1), :, g:g + 1],
                                             yz_bf[:, sl])
                    # out-GEMM into an SBUF-staged quarter partial (no per-
                    # bundle DMA: avoids DMA-queue contention with the RS)
                    pq = oevp.tile([128, TTQ * D], BF16, tag="pq", name="pq", bufs=3)
                    for tt in range(TTQ):
                        for nch in range(NCH):
                            ps = pso.tile([128, NOUT_CH], F32, tag="pso", name="pso")
                            gtt = h * TTQ + tt
                            for g in range(G6):
                                nc.tensor.matmul(
                                    ps[:], yz_bf[:, g * T + gtt * 128: g * T + (gtt + 1) * 128],
                                    wo[g][:, nch * NOUT_CH:(nch + 1) * NOUT_CH],
                                    start=(g == 0), stop=(g == G6 - 1))
                            dst = pq[:, tt * D + nch * NOUT_CH:
                                     tt * D + (nch + 1) * NOUT_CH]
                            if (tt * NCH + nch) % 2 == 0:
                                nc.vector.tensor_copy(dst, ps[:])
                            else:
                                nc.scalar.copy(dst, ps[:])
                    for tt in range(TTQ):
                        nc.sync.dma_start(
                            out=part_b[h][tt * 128:(tt + 1) * 128, :],
                            in_=pq[:, tt * D:(tt + 1) * D])
                    nc.gpsimd.collective_compute(
                        "ReduceScatter", OP.add,
                        ins=[part_b[h][:].opt()], outs=[rs_out[h][:].opt()],
                        replica_groups=[[0, 1, 2, 3], [4, 5, 6, 7]])
                    nc.sync.dma_start(out=outp[h], in_=rs_out[h][:])
            wop.release()

    nc.compile()
    return nc


def _host_prep(inputs):
    h = np.asarray(inputs["hidden_states"], np.float32)
    gamma = np.asarray(inputs["ln_gamma"], np.float32)
    W_in = np.asarray(inputs["W_in"], np.float32)
    W_z = np.asarray(inputs["W_z"], np.float32)
    W_dt = np.asarray(inputs["W_dt"], np.float32)
    b_dt = np.asarray(inputs["b_dt"], np.float32)
    W_B = np.asarray(inputs["W_B"], np.float32)
    W_C = np.asarray(inputs["W_C"], np.float32)
    A_log = np.asarray(inputs["A_log"], np.float32)
    W_out = np.asarray(inputs["W_out"], np.float32)
    v_th_raw = np.asarray(inputs["v_th_raw"], np.float32)

    A = (-np.exp(A_log)).astype(np.float32)                      # (D, N)
    v_th = (V_TH_MIN + np.log1p(np.exp(v_th_raw))).astype(np.float32)
    v_th_d = np.repeat(v_th, D // KG)                            # (D,)
    Wq = {0: gamma[:, None] * W_in, 1: gamma[:, None] * W_z, 2: gamma[:, None] * W_dt}
    WBC = np.concatenate([gamma[:, None] * W_B, gamma[:, None] * W_C], 1)  # (D, 8)

    onesc_h = bf16r(np.ones((128, 1)))
    ones1_h = bf16r(np.ones((1, 128)))
    selmu_h = np.zeros((9, 128), np.float32)
    selmu_h[0, :] = 1.0 / D
    selmu_h = bf16r(selmu_h)
    sel9_h = np.zeros((9, 8 * P112), np.float32)
    for n in range(8):
        sel9_h[n + 1, n * P112:(n + 1) * P112] = 1.0
    sel9_h = bf16r(sel9_h)

    in_maps = []
    for c in range(NCORE):
        b, q4 = c // 4, c % 4
        p = np.arange(P112)
        chs = {g: q4 * QD + 6 * p + g for g in range(G6)}

        # main cat: block b = qty*6+g holds cols b*128 + p (112 real + 16
        # zero-pad); block 18 = [ones | W_B(4) | W_C(4) | pad].
        wcat = np.zeros((D, CATW), np.float32)
        for qty in range(3):
            for g in range(G6):
                bi = qty * G6 + g
                wcat[:, bi * 128:bi * 128 + P112] = Wq[qty][:, chs[g]]
        wcat[:, NB * 128] = 1.0
        wcat[:, NB * 128 + 1:NB * 128 + 9] = WBC
        wcat_bf = wcat.astype(ml_dtypes.bfloat16)
        wcat_f = wcat_bf.astype(np.float32)
        cs = wcat_f.sum(0, dtype=np.float32)
        csmat_h = bf16r(np.tile(-cs[None, :NB * 128] / 128.0, (128, 1)))
        csb9_h = np.zeros((9, 1), np.float32)
        csb9_h[1:9, 0] = -cs[NB * 128 + 1:NB * 128 + 9]
        wdma_all = np.ascontiguousarray(
            wcat_bf.reshape(KT, 128, NB + 1, 128).transpose(2, 1, 0, 3).reshape(NB + 1, 128, KT * 128))
        wdma = np.ascontiguousarray(wdma_all[:NB])
        w18_h = np.ascontiguousarray(wdma_all[NB])

        hTb = bf16r(h[b].T)                                      # (D, T) bf16
        hdma = np.ascontiguousarray(
            hTb.reshape(KT, 128, NTC, TC).transpose(2, 0, 1, 3))

        wout_p = np.empty((G6, P112, D), ml_dtypes.bfloat16)
        for g in range(G6):
            wout_p[g] = bf16r(W_out[chs[g], :])

        aperm = np.empty((P112, G6 * N), np.float32)
        bdtp = np.empty((P112, G6), np.float32)
        for g in range(G6):
            aperm[:, g * N:(g + 1) * N] = A[chs[g], :]
            bdtp[:, g] = b_dt[chs[g]]
        vth_p = v_th_d[chs[0]].astype(np.float32).reshape(P112, 1)

        in_maps.append({
            "hT": hdma, "wcat": wdma, "w18": w18_h, "csmat": csmat_h,
            "csb9": csb9_h, "selmu": selmu_h, "sel9": sel9_h,
            "wout": np.ascontiguousarray(wout_p),
            "aperm": aperm, "vb4": -SPIKE_BETA * vth_p, "vbn": -vth_p,
            "bdt": bdtp, "onesc": onesc_h, "ones1": ones1_h,
            "ones1f": np.ones((1, 128), np.float32),
        })
    return in_maps


def kernel(trace=False, **inputs):
    if "nc" not in _CACHE:
        _CACHE["nc"] = _build()
    nc = _CACHE["nc"]
    in_maps = _host_prep(inputs)
    res = run_bass_kernel_spmd(nc, in_maps, core_ids=list(range(NCORE)), trace=trace)
    out = np.empty((B, T, D), np.float32)
    for c in range(NCORE):
        b, q4 = c // 4, c % 4
        o = np.asarray(res.results[c]["out"], dtype=np.float32)  # (4, 128, D)
        for hh in range(4):
            out[b, hh * (T // 4) + q4 * 128: hh * (T // 4) + (q4 + 1) * 128, :] = o[hh]
    if trace:
        kernel.last_exec_time_ns = res.exec_time_ns
    return out


# revision 26
# speedup vs baseline: 1.0269x; 1.0269x over previous
"""TRN2 Bass kernel for nn_BioSSMMixer.

Sharding: 8 cores = DP over batch (2) x TP over D-channels (4 x 672).
Per core: bf16 cat-GEMM over 18 x 128-wide padded blocks [W_in|W_z|W_dt]
(128-col weights keep FWL) plus a stats/BC block [ones|W_B|W_C] computed a
chunk ahead; the LayerNorm mean-correction is folded into the contraction
as one extra matmul per block against a broadcast mu (lhsT = outer(ones,
-colsum/128)), so the PSUM epilogue is a single rB-scale mul per block;
fp32 tensor_tensor_scan for the SSM state (bf16 forcing); nonlinear
spiking membrane scan runs chunk-parallel over time (32 chunks of 64
steps + 64 warmup steps); ReduceScatter of the partial out-GEMM within
each 4-core group; final token-sharded output.
"""
import sys, types

sys.path.insert(0, "/opt/trn_rl_repo")

# Inject the missing antenv.axon_hooks so trace=True can profile via NTFF.
try:
    import antenv

    if "antenv.axon_hooks" not in sys.modules:
        _m = types.ModuleType("antenv.axon_hooks")
        _m._hook = None

        def _set(h):
            _m._hook = h

        def _get():
            return _m._hook

        _m.set_axon_ntff_profile_hook = _set
        _m.get_axon_ntff_profile_hook = _get
        sys.modules["antenv.axon_hooks"] = _m
        antenv.axon_hooks = _m
        try:
            from trn_agent_boot.trn_boot import _ntff_profile_via_ctypes

            hk = _ntff_profile_via_ctypes("/opt/axon/libaxon_pjrt.so")
            if hk is not None:
                _m._hook = hk
        except Exception:
            pass
except Exception:
    pass

import numpy as np
import ml_dtypes

import concourse.bass as bass
import concourse.mybir as mybir
import concourse.tile as tile
from concourse import bacc
from concourse.bass_utils import run_bass_kernel_spmd

F32 = mybir.dt.float32
BF16 = mybir.dt.bfloat16
AF = mybir.ActivationFunctionType
OP = mybir.AluOpType

# ---- problem constants (hardcoded per the harness contract) ----
D, T, B, N, KG = 2688, 2048, 2, 4, 16
V_TH_MIN, SPIKE_BETA, V_DECAY, LN_EPS = 0.1, 4.0, 0.9, 1e-5
NCORE = 8
QD = D // 4            # 672 channels per core
P112 = 112             # partition rows per g-group
G6 = 6                 # g-groups per core (112*6 = 672)
TC = 512               # time chunk for GEMM/scan phases
NTC = T // TC          # 4
KT = D // 128          # 21 k-tiles
NB = 18                # main cat blocks: (112 real + 16 pad) each
CATW = (NB + 1) * 128  # + stats/BC block [ones|W_B|W_C|pad]
NCHUNK = 64            # membrane scan chunks
LCH = T // NCHUNK      # 64
WARM = 32              # membrane warmup steps = LCH (0.9^32; sim-validated)
NOUT_CH = 448          # out-GEMM N chunk (6*448 = 2688)

bf16r = lambda x: np.ascontiguousarray(np.asarray(x, np.float32).astype(ml_dtypes.bfloat16))

_CACHE = {}


def _build():
    nc = bacc.Bacc("TRN2", target_bir_lowering=False, debug=False, num_devices=NCORE)

    hT = nc.declare_dram_parameter("hT", [NTC, KT, 128, TC], BF16, isOutput=False)
    wcat = nc.declare_dram_parameter("wcat", [NB, 128, KT * 128], BF16, isOutput=False)
    w18 = nc.declare_dram_parameter("w18", [128, KT * 128], BF16, isOutput=False)
    csmat = nc.declare_dram_parameter("csmat", [128, NB * 128], BF16, isOutput=False)
    csb9 = nc.declare_dram_parameter("csb9", [9, 1], F32, isOutput=False)
    selmu = nc.declare_dram_parameter("selmu", [9, 128], BF16, isOutput=False)
    sel9 = nc.declare_dram_parameter("sel9", [9, 8 * P112], BF16, isOutput=False)
    wout = nc.declare_dram_parameter("wout", [G6, P112, D], BF16, isOutput=False)
    aperm = nc.declare_dram_parameter("aperm", [P112, G6 * N], F32, isOutput=False)
    vb4 = nc.declare_dram_parameter("vb4", [P112, 1], F32, isOutput=False)
    vbn = nc.declare_dram_parameter("vbn", [P112, 1], F32, isOutput=False)
    bdt = nc.declare_dram_parameter("bdt", [P112, G6], F32, isOutput=False)
    onesc = nc.declare_dram_parameter("onesc", [128, 1], BF16, isOutput=False)
    ones1 = nc.declare_dram_parameter("ones1", [1, 128], BF16, isOutput=False)
    ones1f = nc.declare_dram_parameter("ones1f", [1, 128], F32, isOutput=False)
    outp = nc.declare_dram_parameter("out", [4, TC // 4, D], BF16, isOutput=True)

    with tile.TileContext(nc) as tc:
        with (
            tc.tile_pool(name="consts", bufs=1) as cpool,
            tc.tile_pool(name="ybuf", bufs=1) as ypool,
            tc.tile_pool(name="dram", bufs=1, space="DRAM") as dpool,
        ):
            # ---- load constants to SBUF ----
            a_sb = cpool.tile([P112, G6 * N], F32)
            vb4_sb = cpool.tile([P112, 1], F32)
            vbn_sb = cpool.tile([P112, 1], F32)
            bdt_sb = cpool.tile([P112, G6], F32)
            cs_sb = cpool.tile([128, NB * 128], BF16)
            csb9_sb = cpool.tile([9, 1], F32)
            selmu_sb = cpool.tile([9, 128], BF16)
            sel9_sb = cpool.tile([9, 8 * P112], BF16)
            onesc_sb = cpool.tile([128, 1], BF16)
            ones1_sb = cpool.tile([1, 128], BF16)
            ones1f_sb = cpool.tile([1, 128], F32)
            w18_sb = cpool.tile([128, KT * 128], BF16)
            for dst, src in [(a_sb, aperm), (vb4_sb, vb4), (vbn_sb, vbn),
                             (bdt_sb, bdt), (cs_sb, csmat), (csb9_sb, csb9),
                             (selmu_sb, selmu), (sel9_sb, sel9),
                             (onesc_sb, onesc), (ones1_sb, ones1),
                             (ones1f_sb, ones1f), (w18_sb, w18)]:
                nc.sync.dma_start(out=dst[:], in_=src[:])

            # persistent big buffers
            y_bf = ypool.tile([P112, G6 * T], BF16)     # y, tau-major time
            yz_bf = ypool.tile([P112, G6 * T], BF16)    # y*z, token-major
            s_carry = cpool.tile([P112, G6 * N], F32)   # scan carries

            with (
                tc.tile_pool(name="ht", bufs=1) as htp,
                tc.tile_pool(name="w", bufs=2) as wp,
                tc.tile_pool(name="sq", bufs=2) as sqp,
                tc.tile_pool(name="udt", bufs=1) as udtp,
                tc.tile_pool(name="zp", bufs=1) as zpool,
                tc.tile_pool(name="scr", bufs=1) as scr,
                tc.tile_pool(name="rows", bufs=1) as rowp,
                tc.tile_pool(name="ps_gemm", bufs=3, space="PSUM") as psg,
                tc.tile_pool(name="ps_st", bufs=1, space="PSUM") as pssq,
                tc.tile_pool(name="ps_bc", bufs=1, space="PSUM") as psbc,
            ):
                def stage_load(tci):
                    """hts, stats/BC block, LN stats, muB/rB broadcasts."""
                    hts = []
                    for k in range(KT):
                        ht_t = htp.tile([128, TC], BF16, tag=f"ht{k}", name=f"ht{k}")
                        nc.sync.dma_start(out=ht_t[:], in_=hT[tci, k])
                        hts.append(ht_t)
                    # block 18: [ones | W_B | W_C] -> sum row + raw Bm/Cm rows
                    ps18 = psbc.tile([128, TC], F32, tag="bc18", name="ps18")
                    for k in range(KT):
                        nc.tensor.matmul(ps18[:], w18_sb[:, k * 128:(k + 1) * 128],
                                         hts[k][:], start=(k == 0), stop=(k == KT - 1))
                    raw = rowp.tile([9, TC], BF16, tag=f"raw{tci % 2}", name="raw")
                    nc.vector.tensor_copy(raw[:], ps18[0:9, :])
                    # sumsq via Square + ones-column reduce
                    ps_sq = pssq.tile([1, TC], F32, tag="psq", name="psq")
                    for k in range(KT):
                        sq_t = sqp.tile([128, TC], BF16, tag=f"sq{k % 2}")
                        nc.scalar.activation(sq_t[:], hts[k][:], AF.Square)
                        nc.tensor.matmul(ps_sq[:], onesc_sb[:], sq_t[:],
                                         start=(k == 0), stop=(k == KT - 1))
                    # mu broadcast to all 128 partitions (bf16)
                    mps = psbc.tile([128, TC], F32, tag="bc", bufs=2)
                    nc.tensor.matmul(mps[:], selmu_sb[:], raw[:])
                    muB = scr.tile([128, TC], BF16, tag=f"muB{tci % 2}", name="muB")
                    nc.vector.tensor_copy(muB[:], mps[:])
                    # mean-corrected raw BC rows: rawc = raw - mu*colsum
                    rawc = rowp.tile([9, TC], BF16, tag=f"rawc{tci % 2}", name="rawc")
                    nc.vector.scalar_tensor_tensor(rawc[:], muB[0:9, :],
                                                   csb9_sb[:, 0:1], raw[:],
                                                   OP.mult, OP.add)
                    m2 = rowp.tile([1, TC], F32, tag="m2")
                    nc.vector.tensor_mul(m2[:], muB[0:1, :], muB[0:1, :])
                    var = rowp.tile([1, TC], F32, tag="var")
                    nc.vector.scalar_tensor_tensor(var[:], ps_sq[:], 1.0 / D,
                                                   m2[:], OP.mult, OP.subtract)
                    vare = rowp.tile([1, TC], F32, tag="m2")
                    nc.vector.tensor_scalar_add(vare[:], var[:], LN_EPS)
                    sd = rowp.tile([1, TC], F32, tag="sd")
                    nc.scalar.activation(sd[:], vare[:], AF.Sqrt)
                    r = rowp.tile([1, TC], F32, tag="var")
                    nc.vector.reciprocal_approx_fast(out=r[:], in_=sd[:])
                    rps = psbc.tile([128, TC], F32, tag="bc", bufs=2)
                    nc.tensor.matmul(rps[:], ones1f_sb[:], r[:])
                    rB = scr.tile([128, TC], F32, tag=f"rB{tci % 2}", name="rB")
                    nc.vector.tensor_copy(rB[:], rps[:])
                    return hts, rB, muB, rawc

                yv = y_bf[:].rearrange("p (tau c g) -> p c tau g",
                                       tau=LCH, c=NCHUNK, g=G6)
                CPT = TC // LCH

                def scan_core(tci, g, u_t, dt_t, BmB, CmB):
                    # SSM state scan + y for one g-group (yz deferred until z)
                    du = scr.tile([P112, TC], BF16, tag="du")
                    nc.vector.tensor_mul(du[:], dt_t[g][:], u_t[g][:])
                    s_of_n = []
                    for n in range(N):
                        dec = scr.tile([P112, TC], F32, tag=f"dec{n}")
                        nc.scalar.activation(dec[:], dt_t[g][:], AF.Exp,
                                             scale=a_sb[:, g * N + n:g * N + n + 1])
                        inp = scr.tile([P112, TC], BF16, tag=f"inp{n % 2}")
                        nc.vector.tensor_mul(inp[:], du[:], BmB[n][:])
                        s_t = scr.tile([P112, TC], F32, tag=f"s{n}")
                        ini = 0.0 if tci == 0 else s_carry[:, g * N + n:g * N + n + 1]
                        nc.vector.tensor_tensor_scan(s_t[:], dec[:], inp[:], ini,
                                                     OP.mult, OP.add)
                        nc.vector.tensor_copy(s_carry[:, g * N + n:g * N + n + 1],
                                              s_t[:, TC - 1:TC])
                        s_of_n.append(s_t)
                    yac = scr.tile([P112, TC], F32, tag="yac")
                    tmp = scr.tile([P112, TC], F32, tag="ytmp")
                    nc.vector.tensor_mul(yac[:], s_of_n[0][:], CmB[0][:])
                    nc.vector.tensor_mul(tmp[:], s_of_n[1][:], CmB[1][:])
                    nc.gpsimd.tensor_add(yac[:], yac[:], tmp[:])
                    nc.vector.tensor_mul(tmp[:], s_of_n[2][:], CmB[2][:])
                    nc.gpsimd.tensor_add(yac[:], yac[:], tmp[:])
                    nc.vector.tensor_mul(tmp[:], s_of_n[3][:], CmB[3][:])
                    ysl = yv[:, CPT * tci:CPT * (tci + 1), :, g:g + 1]
                    nc.vector.tensor_add(ysl, yac[:], tmp[:])

                def gemm_block(jt, hts, muB, rB, dst):
                    wt = wp.tile([128, KT * 128], BF16, tag="w")
                    nc.sync.dma_start(out=wt[:], in_=wcat[jt])
                    ps = psg.tile([128, TC], F32, tag="psg")
                    for k in range(KT):
                        nc.tensor.matmul(ps[:], wt[:, k * 128:(k + 1) * 128],
                                         hts[k][:], start=(k == 0), stop=False)
                    nc.tensor.matmul(ps[:], cs_sb[:, jt * 128:(jt + 1) * 128],
                                     muB[:], start=False, stop=True)
                    nc.vector.tensor_mul(dst[:], ps[0:P112, :], rB[0:P112, :])

                staged = {0: stage_load(0)}
                for tci in range(NTC):
                    hts, rB, muB, rawc = staged.pop(tci)
                    u_t = {g: udtp.tile([P112, TC], BF16, tag=f"u{g}", name=f"u{g}") for g in range(G6)}
                    dt_t = {g: udtp.tile([P112, TC], BF16, tag=f"dt{g}", name=f"dtt{g}") for g in range(G6)}
                    zpre = {g: zpool.tile([P112, TC], BF16, tag=f"zp{g}", name=f"zpre{g}") for g in range(G6)}
                    dpre = {g: zpool.tile([P112, TC], F32, tag=f"dp{g}", name=f"dpre{g}") for g in range(G6)}
                    # Bm/Cm broadcasts first (rB-scale folded into the evac mul)
                    BmB, CmB = {}, {}
                    for n in range(2 * N):
                        b_ps = psbc.tile([P112, TC], F32, tag="bc", bufs=2)
                        nc.tensor.matmul(b_ps[:], sel9_sb[:, n * P112:(n + 1) * P112],
                                         rawc[:])
                        b_sb = scr.tile([P112, TC], BF16, tag=f"bc{n}_{tci % 2}", name=f"bc{n}")
                        nc.vector.tensor_mul(b_sb[:], b_ps[:], rB[0:P112, :])
                        (BmB if n < N else CmB)[n % N] = b_sb
                    # g-pair-batched u/dt blocks with this chunk's scans
                    # interleaved; pairing halves the ACT table reloads.
                    for gp in range(0, G6, 2):
                        e_t = {}
                        for g in (gp, gp + 1):
                            gemm_block(g, hts, muB, rB, u_t[g])
                            gemm_block(12 + g, hts, muB, rB, dpre[g])
                        for g in (gp, gp + 1):
                            e_t[g] = zpool.tile([P112, TC], BF16, tag=f"e{g % 2}", name=f"et{g}")
                            nc.scalar.activation(e_t[g][:], dpre[g][:], AF.Exp,
                                                 bias=bdt_sb[:, g:g + 1])
                        for g in (gp, gp + 1):
                            nc.scalar.activation(dt_t[g][:], e_t[g][:], AF.Ln, bias=1.0)
                        for g in (gp, gp + 1):
                            scan_core(tci, g, u_t, dt_t, BmB, CmB)
                    # z blocks + sigmoid + yz
                    for g in range(G6):
                        gemm_block(6 + g, hts, muB, rB, zpre[g])
                    z_t = {g: zpool.tile([P112, TC], BF16, tag=f"z{g}", name=f"zt{g}") for g in range(G6)}
                    for g in range(G6):
                        nc.scalar.activation(z_t[g][:], zpre[g][:], AF.Sigmoid)
                    for g in range(G6):
                        eng = nc.gpsimd if g % 2 == 0 else nc.vector
                        eng.tensor_mul(
                            yz_bf[:, g * T + tci * TC: g * T + (tci + 1) * TC],
                            yv[:, CPT * tci:CPT * (tci + 1), :, g:g + 1], z_t[g][:])

                    # prefetch next chunk's stats
                    if tci + 1 < NTC:
                        staged[tci + 1] = stage_load(tci + 1)

            # ========== membrane scan + overlapped tail ==========
            wop = tc.alloc_tile_pool(name="wo", bufs=1)
            wo = []
            for g in range(G6):
                wt = wop.tile([P112, D], BF16, tag=f"wo{g}", name=f"wo{g}")
                nc.sync.dma_start(out=wt[:], in_=wout[g])
                wo.append(wt)
            NQ = 4
            HT2 = T // NQ          # 512 tokens per quarter
            part_b = [dpool.tile([HT2, D], BF16, name=f"partb{h}") for h in range(NQ)]
            rs_out = [dpool.tile([HT2 // 4, D], BF16, name=f"rsout{h}") for h in range(NQ)]
            WAL = NCHUNK * G6          # columns per tau row
            CHAINS = [(0, 22), (22, 22), (44, 20)]   # (c_lo, n_chunks) per chain
            with (
                tc.tile_pool(name="spk", bufs=1) as spp,
                tc.tile_pool(name="vv", bufs=1) as vvp,
                tc.tile_pool(name="vpre", bufs=3) as vpp,
                tc.tile_pool(name="oev", bufs=1) as oevp,
                tc.tile_pool(name="ps_o", bufs=4, space="PSUM") as pso,
            ):
                # spike buffer shares y_bf's tau-major layout: free = tau*192 + c*6 + g
                sp_bf = spp.tile([P112, G6 * T], BF16, name="spbf")
                spv = sp_bf[:].rearrange("p (tau c g) -> p c tau g",
                                         tau=LCH, c=NCHUNK, g=G6)
                v_c, spw = {}, {}
                for x, (c_lo, ncc) in enumerate(CHAINS):
                    hcx = ncc * G6
                    v_c[x] = vvp.tile([P112, hcx], BF16, tag=f"v{x}", name=f"v{x}")
                    spw[x] = vvp.tile([P112, hcx], BF16, tag=f"sw{x}", name=f"sw{x}")
                    nc.vector.memset(v_c[x][:], 0.0)

                def vstep(tau, warm, x):
                    c_lo, ncc = CHAINS[x]
                    hcx = ncc * G6
                    if warm:
                        lo = max(c_lo, 1)          # chunk 0 has no warmup
                        vs = v_c[x][:, (lo - c_lo) * G6:hcx]
                        yo = (LCH + tau) * WAL + (lo - 1) * G6
                        wdt = (c_lo + ncc - lo) * G6
                        sps = spw[x][:, (lo - c_lo) * G6:hcx]
                    else:
                        vs = v_c[x][:, 0:hcx]
                        yo = tau * WAL + c_lo * G6
                        wdt = hcx
                        sps = sp_bf[:, tau * WAL + c_lo * G6:
                                    tau * WAL + c_lo * G6 + hcx]
                    ys = y_bf[:, yo:yo + wdt]
                    vp = vpp.tile([P112, 22 * G6], BF16, tag=f"vp{x}", name=f"vp{x}")
                    vps = vp[:, 0:wdt]
                    nc.vector.scalar_tensor_tensor(vps, vs, V_DECAY, ys, OP.mult, OP.add)
                    nc.scalar.activation(sps, vps, AF.Sigmoid,
                                         bias=vb4_sb[:, 0:1], scale=SPIKE_BETA)
                    nc.vector.scalar_tensor_tensor(vs, sps, vbn_sb[:, 0:1], vps,
                                                   OP.mult, OP.add)

                for tau in range(-WARM, 0):
                    for x in range(len(CHAINS)):
                        vstep(tau, True, x)
                for tau in range(LCH):
                    for x in range(len(CHAINS)):
                        vstep(tau, False, x)

                CPT = TC // LCH
                NCH = D // NOUT_CH
                TTQ = HT2 // 128

                for h in range(NQ):
                    # g = spike * (y*z) for this quarter only (keep gpsimd
                    # clear of the collective queue)
                    for g in range(G6):
                        sl = slice(g * T + h * TC, g * T + (h + 1) * TC)
                        nc.vector.tensor_mul(yz_bf[:, sl],
                                             spv[:, CPT * h:CPT * (h + 1), :, g:g + 1],
                                             yz_bf[:, sl])
                    # out-GEMM into an SBUF-staged quarter partial (no per-
                    # bundle DMA: avoids DMA-queue contention with the RS)
                    pq = oevp.tile([128, TTQ * D], BF16, tag="pq", name="pq", bufs=3)
                    for tt in range(TTQ):
                        for nch in range(NCH):
                            ps = pso.tile([128, NOUT_CH], F32, tag="pso", name="pso")
                            gtt = h * TTQ + tt
                            for g in range(G6):
                                nc.tensor.matmul(
                                    ps[:], yz_bf[:, g * T + gtt * 128: g * T + (gtt + 1) * 128],
                                    wo[g][:, nch * NOUT_CH:(nch + 1) * NOUT_CH],
                                    start=(g == 0), stop=(g == G6 - 1))
                            dst = pq[:, tt * D + nch * NOUT_CH:
                                     tt * D + (nch + 1) * NOUT_CH]
                            if (tt * NCH + nch) % 2 == 0:
                                nc.vector.tensor_copy(dst, ps[:])
                            else:
                                nc.scalar.copy(dst, ps[:])
                    for tt in range(TTQ):
                        nc.sync.dma_start(
                            out=part_b[h][tt * 128:(tt + 1) * 128, :],
                            in_=pq[:, tt * D:(tt + 1) * D])
                    nc.gpsimd.collective_compute(
                        "ReduceScatter", OP.add,
                        ins=[part_b[h][:].opt()], outs=[rs_out[h][:].opt()],
                        replica_groups=[[0, 1, 2, 3], [4, 5, 6, 7]])
                    nc.sync.dma_start(out=outp[h], in_=rs_out[h][:])
            wop.release()

    nc.compile()
    return nc


def _host_prep(inputs):
    h = np.asarray(inputs["hidden_states"], np.float32)
    gamma = np.asarray(inputs["ln_gamma"], np.float32)
    W_in = np.asarray(inputs["W_in"], np.float32)
    W_z = np.asarray(inputs["W_z"], np.float32)
    W_dt = np.asarray(inputs["W_dt"], np.float32)
    b_dt = np.asarray(inputs["b_dt"], np.float32)
    W_B = np.asarray(inputs["W_B"], np.float32)
    W_C = np.asarray(inputs["W_C"], np.float32)
    A_log = np.asarray(inputs["A_log"], np.float32)
    W_out = np.asarray(inputs["W_out"], np.float32)
    v_th_raw = np.asarray(inputs["v_th_raw"], np.float32)

    A = (-np.exp(A_log)).astype(np.float32)                      # (D, N)
    v_th = (V_TH_MIN + np.log1p(np.exp(v_th_raw))).astype(np.float32)
    v_th_d = np.repeat(v_th, D // KG)                            # (D,)
    Wq = {0: gamma[:, None] * W_in, 1: gamma[:, None] * W_z, 2: gamma[:, None] * W_dt}
    WBC = np.concatenate([gamma[:, None] * W_B, gamma[:, None] * W_C], 1)  # (D, 8)

    onesc_h = bf16r(np.ones((128, 1)))
    ones1_h = bf16r(np.ones((1, 128)))
    selmu_h = np.zeros((9, 128), np.float32)
    selmu_h[0, :] = 1.0 / D
    selmu_h = bf16r(selmu_h)
    sel9_h = np.zeros((9, 8 * P112), np.float32)
    for n in range(8):
        sel9_h[n + 1, n * P112:(n + 1) * P112] = 1.0
    sel9_h = bf16r(sel9_h)

    in_maps = []
    for c in range(NCORE):
        b, q4 = c // 4, c % 4
        p = np.arange(P112)
        chs = {g: q4 * QD + 6 * p + g for g in range(G6)}

        # main cat: block b = qty*6+g holds cols b*128 + p (112 real + 16
        # zero-pad); block 18 = [ones | W_B(4) | W_C(4) | pad].
        wcat = np.zeros((D, CATW), np.float32)
        for qty in range(3):
            for g in range(G6):
                bi = qty * G6 + g
                wcat[:, bi * 128:bi * 128 + P112] = Wq[qty][:, chs[g]]
        wcat[:, NB * 128] = 1.0
        wcat[:, NB * 128 + 1:NB * 128 + 9] = WBC
        wcat_bf = wcat.astype(ml_dtypes.bfloat16)
        wcat_f = wcat_bf.astype(np.float32)
        cs = wcat_f.sum(0, dtype=np.float32)
        csmat_h = bf16r(np.tile(-cs[None, :NB * 128] / 128.0, (128, 1)))
        csb9_h = np.zeros((9, 1), np.float32)
        csb9_h[1:9, 0] = -cs[NB * 128 + 1:NB * 128 + 9]
        wdma_all = np.ascontiguousarray(
            wcat_bf.reshape(KT, 128, NB + 1, 128).transpose(2, 1, 0, 3).reshape(NB + 1, 128, KT * 128))
        wdma = np.ascontiguousarray(wdma_all[:NB])
        w18_h = np.ascontiguousarray(wdma_all[NB])

        hTb = bf16r(h[b].T)                                      # (D, T) bf16
        hdma = np.ascontiguousarray(
            hTb.reshape(KT, 128, NTC, TC).transpose(2, 0, 1, 3))

        wout_p = np.empty((G6, P112, D), ml_dtypes.bfloat16)
        for g in range(G6):
            wout_p[g] = bf16r(W_out[chs[g], :])

        aperm = np.empty((P112, G6 * N), np.float32)
        bdtp = np.empty((P112, G6), np.float32)
        for g in range(G6):
            aperm[:, g * N:(g + 1) * N] = A[chs[g], :]
            bdtp[:, g] = b_dt[chs[g]]
        vth_p = v_th_d[chs[0]].astype(np.float32).reshape(P112, 1)

        in_maps.append({
            "hT": hdma, "wcat": wdma, "w18": w18_h, "csmat": csmat_h,
            "csb9": csb9_h, "selmu": selmu_h, "sel9": sel9_h,
            "wout": np.ascontiguousarray(wout_p),
            "aperm": aperm, "vb4": -SPIKE_BETA * vth_p, "vbn": -vth_p,
            "bdt": bdtp, "onesc": onesc_h, "ones1": ones1_h,
            "ones1f": np.ones((1, 128), np.float32),
        })
    return in_maps


def kernel(trace=False, **inputs):
    if "nc" not in _CACHE:
        _CACHE["nc"] = _build()
    nc = _CACHE["nc"]
    in_maps = _host_prep(inputs)
    res = run_bass_kernel_spmd(nc, in_maps, core_ids=list(range(NCORE)), trace=trace)
    out = np.empty((B, T, D), np.float32)
    for c in range(NCORE):
        b, q4 = c // 4, c % 4
        o = np.asarray(res.results[c]["out"], dtype=np.float32)  # (4, 128, D)
        for hh in range(4):
            out[b, hh * (T // 4) + q4 * 128: hh * (T // 4) + (q4 + 1) * 128, :] = o[hh]
    if trace:
        kernel.last_exec_time_ns = res.exec_time_ns
    return out


# revision 27
# speedup vs baseline: 1.0819x; 1.0535x over previous
"""TRN2 Bass kernel for nn_BioSSMMixer.

Sharding: 8 cores = DP over batch (2) x TP over D-channels (4 x 672).
Per core: bf16 cat-GEMM over 18 x 128-wide padded blocks [W_in|W_z|W_dt]
(128-col weights keep FWL) plus a stats/BC block [ones|W_B|W_C] computed a
chunk ahead; the LayerNorm mean-correction is folded into the contraction
as one extra matmul per block against a broadcast mu (lhsT = outer(ones,
-colsum/128)), so the PSUM epilogue is a single rB-scale mul per block;
fp32 tensor_tensor_scan for the SSM state (bf16 forcing); nonlinear
spiking membrane scan runs chunk-parallel over time (32 chunks of 64
steps + 64 warmup steps); ReduceScatter of the partial out-GEMM within
each 4-core group; final token-sharded output.
"""
import sys, types

sys.path.insert(0, "/opt/trn_rl_repo")

# Inject the missing antenv.axon_hooks so trace=True can profile via NTFF.
try:
    import antenv

    if "antenv.axon_hooks" not in sys.modules:
        _m = types.ModuleType("antenv.axon_hooks")
        _m._hook = None

        def _set(h):
            _m._hook = h

        def _get():
            return _m._hook

        _m.set_axon_ntff_profile_hook = _set
        _m.get_axon_ntff_profile_hook = _get
        sys.modules["antenv.axon_hooks"] = _m
        antenv.axon_hooks = _m
        try:
            from trn_agent_boot.trn_boot import _ntff_profile_via_ctypes

            hk = _ntff_profile_via_ctypes("/opt/axon/libaxon_pjrt.so")
            if hk is not None:
                _m._hook = hk
        except Exception:
            pass
except Exception:
    pass

import numpy as np
import ml_dtypes

import concourse.bass as bass
import concourse.mybir as mybir
import concourse.tile as tile
from concourse import bacc
from concourse.bass_utils import run_bass_kernel_spmd

F32 = mybir.dt.float32
BF16 = mybir.dt.bfloat16
AF = mybir.ActivationFunctionType
OP = mybir.AluOpType

# ---- problem constants (hardcoded per the harness contract) ----
D, T, B, N, KG = 2688, 2048, 2, 4, 16
V_TH_MIN, SPIKE_BETA, V_DECAY, LN_EPS = 0.1, 4.0, 0.9, 1e-5
NCORE = 8
QD = D // 4            # 672 channels per core
P112 = 112             # partition rows per g-group
G6 = 6                 # g-groups per core (112*6 = 672)
TC = 512               # time chunk for GEMM/scan phases
NTC = T // TC          # 4
KT = D // 128          # 21 k-tiles
NB = 18                # main cat blocks: (112 real + 16 pad) each
CATW = (NB + 1) * 128  # + stats/BC block [ones|W_B|W_C|pad]
NCHUNK = 64            # membrane scan chunks
LCH = T // NCHUNK      # 64
WARM = 32              # membrane warmup steps = LCH (0.9^32; sim-validated)
NOUT_CH = 448          # out-GEMM N chunk (6*448 = 2688)

bf16r = lambda x: np.ascontiguousarray(np.asarray(x, np.float32).astype(ml_dtypes.bfloat16))

_CACHE = {}


def _build():
    nc = bacc.Bacc("TRN2", target_bir_lowering=False, debug=False, num_devices=NCORE)

    hT = nc.declare_dram_parameter("hT", [NTC, KT, 128, TC], BF16, isOutput=False)
    wcat = nc.declare_dram_parameter("wcat", [NB, 128, KT * 128], BF16, isOutput=False)
    w18 = nc.declare_dram_parameter("w18", [128, KT * 128], BF16, isOutput=False)
    csmat = nc.declare_dram_parameter("csmat", [128, NB * 128], BF16, isOutput=False)
    csb9 = nc.declare_dram_parameter("csb9", [9, 1], F32, isOutput=False)
    selmu = nc.declare_dram_parameter("selmu", [9, 128], BF16, isOutput=False)
    sel9 = nc.declare_dram_parameter("sel9", [9, 8 * P112], BF16, isOutput=False)
    wout = nc.declare_dram_parameter("wout", [G6, P112, D], BF16, isOutput=False)
    aperm = nc.declare_dram_parameter("aperm", [P112, G6 * N], F32, isOutput=False)
    vb4 = nc.declare_dram_parameter("vb4", [P112, 1], F32, isOutput=False)
    vbn = nc.declare_dram_parameter("vbn", [P112, 1], F32, isOutput=False)
    bdt = nc.declare_dram_parameter("bdt", [P112, G6], F32, isOutput=False)
    onesc = nc.declare_dram_parameter("onesc", [128, 1], BF16, isOutput=False)
    ones1 = nc.declare_dram_parameter("ones1", [1, 128], BF16, isOutput=False)
    ones1f = nc.declare_dram_parameter("ones1f", [1, 128], F32, isOutput=False)
    outp = nc.declare_dram_parameter("out", [4, TC // 4, D], BF16, isOutput=True)

    with tile.TileContext(nc) as tc:
        with (
            tc.tile_pool(name="consts", bufs=1) as cpool,
            tc.tile_pool(name="ybuf", bufs=1) as ypool,
            tc.tile_pool(name="dram", bufs=1, space="DRAM") as dpool,
        ):
            # ---- load constants to SBUF ----
            a_sb = cpool.tile([P112, G6 * N], F32)
            vb4_sb = cpool.tile([P112, 1], F32)
            vbn_sb = cpool.tile([P112, 1], F32)
            bdt_sb = cpool.tile([P112, G6], F32)
            cs_sb = cpool.tile([128, NB * 128], BF16)
            csb9_sb = cpool.tile([9, 1], F32)
            selmu_sb = cpool.tile([9, 128], BF16)
            sel9_sb = cpool.tile([9, 8 * P112], BF16)
            onesc_sb = cpool.tile([128, 1], BF16)
            ones1_sb = cpool.tile([1, 128], BF16)
            ones1f_sb = cpool.tile([1, 128], F32)
            w18_sb = cpool.tile([128, KT * 128], BF16)
            for dst, src in [(a_sb, aperm), (vb4_sb, vb4), (vbn_sb, vbn),
                             (bdt_sb, bdt), (cs_sb, csmat), (csb9_sb, csb9),
                             (selmu_sb, selmu), (sel9_sb, sel9),
                             (onesc_sb, onesc), (ones1_sb, ones1),
                             (ones1f_sb, ones1f), (w18_sb, w18)]:
                nc.sync.dma_start(out=dst[:], in_=src[:])

            # persistent big buffers
            y_bf = ypool.tile([P112, G6 * T], BF16)     # y, tau-major time
            yz_bf = ypool.tile([P112, G6 * T], BF16)    # y*z, token-major
            s_carry = cpool.tile([P112, G6 * N], F32)   # scan carries

            with (
                tc.tile_pool(name="ht", bufs=1) as htp,
                tc.tile_pool(name="w", bufs=3) as wp,
                tc.tile_pool(name="sq", bufs=2) as sqp,
                tc.tile_pool(name="udt", bufs=1) as udtp,
                tc.tile_pool(name="zp", bufs=1) as zpool,
                tc.tile_pool(name="scr", bufs=1) as scr,
                tc.tile_pool(name="rows", bufs=1) as rowp,
                tc.tile_pool(name="ps_gemm", bufs=4, space="PSUM") as psg,
                tc.tile_pool(name="ps_st", bufs=1, space="PSUM") as pssq,
                tc.tile_pool(name="ps_bc", bufs=1, space="PSUM") as psbc,
            ):
                def stage_load(tci):
                    """hts, stats/BC block, LN stats, muB/rB broadcasts."""
                    hts = []
                    for k in range(KT):
                        ht_t = htp.tile([128, TC], BF16, tag=f"ht{k}", name=f"ht{k}")
                        nc.sync.dma_start(out=ht_t[:], in_=hT[tci, k])
                        hts.append(ht_t)
                    # block 18: [ones | W_B | W_C] -> sum row + raw Bm/Cm rows
                    ps18 = psbc.tile([128, TC], F32, tag="bc18", name="ps18")
                    for k in range(KT):
                        nc.tensor.matmul(ps18[:], w18_sb[:, k * 128:(k + 1) * 128],
                                         hts[k][:], start=(k == 0), stop=(k == KT - 1))
                    raw = rowp.tile([9, TC], BF16, tag=f"raw{tci % 2}", name="raw")
                    nc.vector.tensor_copy(raw[:], ps18[0:9, :])
                    # sumsq via Square + ones-column reduce
                    ps_sq = pssq.tile([1, TC], F32, tag="psq", name="psq")
                    for k in range(KT):
                        sq_t = sqp.tile([128, TC], BF16, tag=f"sq{k % 2}")
                        nc.scalar.activation(sq_t[:], hts[k][:], AF.Square)
                        nc.tensor.matmul(ps_sq[:], onesc_sb[:], sq_t[:],
                                         start=(k == 0), stop=(k == KT - 1))
                    # mu broadcast to all 128 partitions (bf16)
                    mps = psbc.tile([128, TC], F32, tag="bc", bufs=2)
                    nc.tensor.matmul(mps[:], selmu_sb[:], raw[:])
                    muB = scr.tile([128, TC], BF16, tag=f"muB{tci % 2}", name="muB")
                    nc.vector.tensor_copy(muB[:], mps[:])
                    # mean-corrected raw BC rows: rawc = raw - mu*colsum
                    rawc = rowp.tile([9, TC], BF16, tag=f"rawc{tci % 2}", name="rawc")
                    nc.vector.scalar_tensor_tensor(rawc[:], muB[0:9, :],
                                                   csb9_sb[:, 0:1], raw[:],
                                                   OP.mult, OP.add)
                    m2 = rowp.tile([1, TC], F32, tag="m2")
                    nc.vector.tensor_mul(m2[:], muB[0:1, :], muB[0:1, :])
                    var = rowp.tile([1, TC], F32, tag="var")
                    nc.vector.scalar_tensor_tensor(var[:], ps_sq[:], 1.0 / D,
                                                   m2[:], OP.mult, OP.subtract)
                    vare = rowp.tile([1, TC], F32, tag="m2")
                    nc.vector.tensor_scalar_add(vare[:], var[:], LN_EPS)
                    sd = rowp.tile([1, TC], F32, tag="sd")
                    nc.scalar.activation(sd[:], vare[:], AF.Sqrt)
                    r = rowp.tile([1, TC], F32, tag="var")
                    nc.vector.reciprocal_approx_fast(out=r[:], in_=sd[:])
                    rps = psbc.tile([128, TC], F32, tag="bc", bufs=2)
                    nc.tensor.matmul(rps[:], ones1f_sb[:], r[:])
                    rB = scr.tile([128, TC], F32, tag=f"rB{tci % 2}", name="rB")
                    nc.vector.tensor_copy(rB[:], rps[:])
                    return hts, rB, muB, rawc

                yv = y_bf[:].rearrange("p (tau c g) -> p c tau g",
                                       tau=LCH, c=NCHUNK, g=G6)
                CPT = TC // LCH

                def scan_core(tci, g, u_t, dt_t, BmB, CmB):
                    # SSM state scan + y for one g-group (yz deferred until z)
                    du = scr.tile([P112, TC], BF16, tag="du")
                    nc.vector.tensor_mul(du[:], dt_t[g][:], u_t[g][:])
                    s_of_n = []
                    for n in range(N):
                        dec = scr.tile([P112, TC], F32, tag=f"dec{n}")
                        nc.scalar.activation(dec[:], dt_t[g][:], AF.Exp,
                                             scale=a_sb[:, g * N + n:g * N + n + 1])
                        inp = scr.tile([P112, TC], BF16, tag=f"inp{n % 2}")
                        nc.vector.tensor_mul(inp[:], du[:], BmB[n][:])
                        s_t = scr.tile([P112, TC], F32, tag=f"s{n}")
                        ini = 0.0 if tci == 0 else s_carry[:, g * N + n:g * N + n + 1]
                        nc.vector.tensor_tensor_scan(s_t[:], dec[:], inp[:], ini,
                                                     OP.mult, OP.add)
                        nc.vector.tensor_copy(s_carry[:, g * N + n:g * N + n + 1],
                                              s_t[:, TC - 1:TC])
                        s_of_n.append(s_t)
                    yac = scr.tile([P112, TC], F32, tag="yac")
                    tmp = scr.tile([P112, TC], F32, tag="ytmp")
                    nc.vector.tensor_mul(yac[:], s_of_n[0][:], CmB[0][:])
                    nc.vector.tensor_mul(tmp[:], s_of_n[1][:], CmB[1][:])
                    nc.gpsimd.tensor_add(yac[:], yac[:], tmp[:])
                    nc.vector.tensor_mul(tmp[:], s_of_n[2][:], CmB[2][:])
                    nc.gpsimd.tensor_add(yac[:], yac[:], tmp[:])
                    nc.vector.tensor_mul(tmp[:], s_of_n[3][:], CmB[3][:])
                    ysl = yv[:, CPT * tci:CPT * (tci + 1), :, g:g + 1]
                    nc.vector.tensor_add(ysl, yac[:], tmp[:])

                def gemm_block(jt, hts, muB, rB, dst):
                    wt = wp.tile([128, KT * 128], BF16, tag="w")
                    nc.sync.dma_start(out=wt[:], in_=wcat[jt])
                    ps = psg.tile([128, TC], F32, tag="psg")
                    for k in range(KT):
                        nc.tensor.matmul(ps[:], wt[:, k * 128:(k + 1) * 128],
                                         hts[k][:], start=(k == 0), stop=False)
                    nc.tensor.matmul(ps[:], cs_sb[:, jt * 128:(jt + 1) * 128],
                                     muB[:], start=False, stop=True)
                    nc.vector.tensor_mul(dst[:], ps[0:P112, :], rB[0:P112, :])

                staged = {0: stage_load(0)}
                for tci in range(NTC):
                    hts, rB, muB, rawc = staged.pop(tci)
                    u_t = {g: udtp.tile([P112, TC], BF16, tag=f"u{g}", name=f"u{g}") for g in range(G6)}
                    dt_t = {g: udtp.tile([P112, TC], BF16, tag=f"dt{g}", name=f"dtt{g}") for g in range(G6)}
                    zpre = {g: zpool.tile([P112, TC], BF16, tag=f"zp{g}", name=f"zpre{g}") for g in range(G6)}
                    dpre = {g: zpool.tile([P112, TC], F32, tag=f"dp{g}", name=f"dpre{g}") for g in range(G6)}
                    # Bm/Cm broadcasts first (rB-scale folded into the evac mul)
                    BmB, CmB = {}, {}
                    for n in range(2 * N):
                        b_ps = psbc.tile([P112, TC], F32, tag="bc", bufs=2)
                        nc.tensor.matmul(b_ps[:], sel9_sb[:, n * P112:(n + 1) * P112],
                                         rawc[:])
                        b_sb = scr.tile([P112, TC], BF16, tag=f"bc{n}_{tci % 2}", name=f"bc{n}")
                        nc.vector.tensor_mul(b_sb[:], b_ps[:], rB[0:P112, :])
                        (BmB if n < N else CmB)[n % N] = b_sb
                    # g-pair-batched u/dt blocks with this chunk's scans
                    # interleaved; pairing halves the ACT table reloads.
                    for gp in range(0, G6, 2):
                        e_t = {}
                        for g in (gp, gp + 1):
                            gemm_block(g, hts, muB, rB, u_t[g])
                            gemm_block(12 + g, hts, muB, rB, dpre[g])
                        for g in (gp, gp + 1):
                            e_t[g] = zpool.tile([P112, TC], BF16, tag=f"e{g % 2}", name=f"et{g}")
                            nc.scalar.activation(e_t[g][:], dpre[g][:], AF.Exp,
                                                 bias=bdt_sb[:, g:g + 1])
                        for g in (gp, gp + 1):
                            nc.scalar.activation(dt_t[g][:], e_t[g][:], AF.Ln, bias=1.0)
                        for g in (gp, gp + 1):
                            scan_core(tci, g, u_t, dt_t, BmB, CmB)
                    # z blocks + sigmoid + yz
                    for g in range(G6):
                        gemm_block(6 + g, hts, muB, rB, zpre[g])
                    z_t = {g: zpool.tile([P112, TC], BF16, tag=f"z{g}", name=f"zt{g}") for g in range(G6)}
                    for g in range(G6):
                        nc.scalar.activation(z_t[g][:], zpre[g][:], AF.Sigmoid)
                    for g in range(G6):
                        eng = nc.gpsimd if g % 2 == 0 else nc.vector
                        eng.tensor_mul(
                            yz_bf[:, g * T + tci * TC: g * T + (tci + 1) * TC],
                            yv[:, CPT * tci:CPT * (tci + 1), :, g:g + 1], z_t[g][:])

                    # prefetch next chunk's stats
                    if tci + 1 < NTC:
                        staged[tci + 1] = stage_load(tci + 1)

            # ========== membrane scan + overlapped tail ==========
            wop = tc.alloc_tile_pool(name="wo", bufs=1)
            wo = []
            for g in range(G6):
                wt = wop.tile([P112, D], BF16, tag=f"wo{g}", name=f"wo{g}")
                nc.sync.dma_start(out=wt[:], in_=wout[g])
                wo.append(wt)
            NQ = 4
            HT2 = T // NQ          # 512 tokens per quarter
            part_b = [dpool.tile([HT2, D], BF16, name=f"partb{h}") for h in range(NQ)]
            rs_out = [dpool.tile([HT2 // 4, D], BF16, name=f"rsout{h}") for h in range(NQ)]
            WAL = NCHUNK * G6          # columns per tau row
            CHAINS = [(0, 22), (22, 22), (44, 20)]   # (c_lo, n_chunks) per chain
            with (
                tc.tile_pool(name="spk", bufs=1) as spp,
                tc.tile_pool(name="vv", bufs=1) as vvp,
                tc.tile_pool(name="vpre", bufs=3) as vpp,
                tc.tile_pool(name="oev", bufs=1) as oevp,
                tc.tile_pool(name="ps_o", bufs=4, space="PSUM") as pso,
            ):
                # spike buffer shares y_bf's tau-major layout: free = tau*192 + c*6 + g
                sp_bf = spp.tile([P112, G6 * T], BF16, name="spbf")
                spv = sp_bf[:].rearrange("p (tau c g) -> p c tau g",
                                         tau=LCH, c=NCHUNK, g=G6)
                v_c, spw = {}, {}
                for x, (c_lo, ncc) in enumerate(CHAINS):
                    hcx = ncc * G6
                    v_c[x] = vvp.tile([P112, hcx], BF16, tag=f"v{x}", name=f"v{x}")
                    spw[x] = vvp.tile([P112, hcx], BF16, tag=f"sw{x}", name=f"sw{x}")
                    nc.vector.memset(v_c[x][:], 0.0)

                def vstep(tau, warm, x):
                    c_lo, ncc = CHAINS[x]
                    hcx = ncc * G6
                    if warm:
                        lo = max(c_lo, 1)          # chunk 0 has no warmup
                        vs = v_c[x][:, (lo - c_lo) * G6:hcx]
                        yo = (LCH + tau) * WAL + (lo - 1) * G6
                        wdt = (c_lo + ncc - lo) * G6
                        sps = spw[x][:, (lo - c_lo) * G6:hcx]
                    else:
                        vs = v_c[x][:, 0:hcx]
                        yo = tau * WAL + c_lo * G6
                        wdt = hcx
                        sps = sp_bf[:, tau * WAL + c_lo * G6:
                                    tau * WAL + c_lo * G6 + hcx]
                    ys = y_bf[:, yo:yo + wdt]
                    vp = vpp.tile([P112, 22 * G6], BF16, tag=f"vp{x}", name=f"vp{x}")
                    vps = vp[:, 0:wdt]
                    nc.vector.scalar_tensor_tensor(vps, vs, V_DECAY, ys, OP.mult, OP.add)
                    nc.scalar.activation(sps, vps, AF.Sigmoid,
                                         bias=vb4_sb[:, 0:1], scale=SPIKE_BETA)
                    nc.vector.scalar_tensor_tensor(vs, sps, vbn_sb[:, 0:1], vps,
                                                   OP.mult, OP.add)

                for tau in range(-WARM, 0):
                    for x in range(len(CHAINS)):
                        vstep(tau, True, x)
                for tau in range(LCH):
                    for x in range(len(CHAINS)):
                        vstep(tau, False, x)

                CPT = TC // LCH
                NCH = D // NOUT_CH
                TTQ = HT2 // 128

                for h in range(NQ):
                    # g = spike * (y*z) for this quarter only (keep gpsimd
                    # clear of the collective queue)
                    for g in range(G6):
                        sl = slice(g * T + h * TC, g * T + (h + 1) * TC)
                        nc.vector.tensor_mul(yz_bf[:, sl],
                                             spv[:, CPT * h:CPT * (h + 1), :, g:g + 1],
                                             yz_bf[:, sl])
                    # out-GEMM into an SBUF-staged quarter partial (no per-
                    # bundle DMA: avoids DMA-queue contention with the RS)
                    pq = oevp.tile([128, TTQ * D], BF16, tag="pq", name="pq", bufs=3)
                    for tt in range(TTQ):
                        for nch in range(NCH):
                            ps = pso.tile([128, NOUT_CH], F32, tag="pso", name="pso")
                            gtt = h * TTQ + tt
                            for g in range(G6):
                                nc.tensor.matmul(
                                    ps[:], yz_bf[:, g * T + gtt * 128: g * T + (gtt + 1) * 128],
                                    wo[g][:, nch * NOUT_CH:(nch + 1) * NOUT_CH],
                                    start=(g == 0), stop=(g == G6 - 1))
                            dst = pq[:, tt * D + nch * NOUT_CH:
                                     tt * D + (nch + 1) * NOUT_CH]
                            if (tt * NCH + nch) % 2 == 0:
                                nc.vector.tensor_copy(dst, ps[:])
                            else:
                                nc.scalar.copy(dst, ps[:])
                    for tt in range(TTQ):
                        nc.sync.dma_start(
                            out=part_b[h][tt * 128:(tt + 1) * 128, :],
                            in_=pq[:, tt * D:(tt + 1) * D])
                    nc.gpsimd.collective_compute(
                        "ReduceScatter", OP.add,
                        ins=[part_b[h][:].opt()], outs=[rs_out[h][:].opt()],
                        replica_groups=[[0, 1, 2, 3], [4, 5, 6, 7]])
                    nc.sync.dma_start(out=outp[h], in_=rs_out[h][:])
            wop.release()

    nc.compile()
    return nc


def _host_prep(inputs):
    h = np.asarray(inputs["hidden_states"], np.float32)
    gamma = np.asarray(inputs["ln_gamma"], np.float32)
    W_in = np.asarray(inputs["W_in"], np.float32)
    W_z = np.asarray(inputs["W_z"], np.float32)
    W_dt = np.asarray(inputs["W_dt"], np.float32)
    b_dt = np.asarray(inputs["b_dt"], np.float32)
    W_B = np.asarray(inputs["W_B"], np.float32)
    W_C = np.asarray(inputs["W_C"], np.float32)
    A_log = np.asarray(inputs["A_log"], np.float32)
    W_out = np.asarray(inputs["W_out"], np.float32)
    v_th_raw = np.asarray(inputs["v_th_raw"], np.float32)

    A = (-np.exp(A_log)).astype(np.float32)                      # (D, N)
    v_th = (V_TH_MIN + np.log1p(np.exp(v_th_raw))).astype(np.float32)
    v_th_d = np.repeat(v_th, D // KG)                            # (D,)
    Wq = {0: gamma[:, None] * W_in, 1: gamma[:, None] * W_z, 2: gamma[:, None] * W_dt}
    WBC = np.concatenate([gamma[:, None] * W_B, gamma[:, None] * W_C], 1)  # (D, 8)

    onesc_h = bf16r(np.ones((128, 1)))
    ones1_h = bf16r(np.ones((1, 128)))
    selmu_h = np.zeros((9, 128), np.float32)
    selmu_h[0, :] = 1.0 / D
    selmu_h = bf16r(selmu_h)
    sel9_h = np.zeros((9, 8 * P112), np.float32)
    for n in range(8):
        sel9_h[n + 1, n * P112:(n + 1) * P112] = 1.0
    sel9_h = bf16r(sel9_h)

    in_maps = []
    for c in range(NCORE):
        b, q4 = c // 4, c % 4
        p = np.arange(P112)
        chs = {g: q4 * QD + 6 * p + g for g in range(G6)}

        # main cat: block b = qty*6+g holds cols b*128 + p (112 real + 16
        # zero-pad); block 18 = [ones | W_B(4) | W_C(4) | pad].
        wcat = np.zeros((D, CATW), np.float32)
        for qty in range(3):
            for g in range(G6):
                bi = qty * G6 + g
                wcat[:, bi * 128:bi * 128 + P112] = Wq[qty][:, chs[g]]
        wcat[:, NB * 128] = 1.0
        wcat[:, NB * 128 + 1:NB * 128 + 9] = WBC
        wcat_bf = wcat.astype(ml_dtypes.bfloat16)
        wcat_f = wcat_bf.astype(np.float32)
        cs = wcat_f.sum(0, dtype=np.float32)
        csmat_h = bf16r(np.tile(-cs[None, :NB * 128] / 128.0, (128, 1)))
        csb9_h = np.zeros((9, 1), np.float32)
        csb9_h[1:9, 0] = -cs[NB * 128 + 1:NB * 128 + 9]
        wdma_all = np.ascontiguousarray(
            wcat_bf.reshape(KT, 128, NB + 1, 128).transpose(2, 1, 0, 3).reshape(NB + 1, 128, KT * 128))
        wdma = np.ascontiguousarray(wdma_all[:NB])
        w18_h = np.ascontiguousarray(wdma_all[NB])

        hTb = bf16r(h[b].T)                                      # (D, T) bf16
        hdma = np.ascontiguousarray(
            hTb.reshape(KT, 128, NTC, TC).transpose(2, 0, 1, 3))

        wout_p = np.empty((G6, P112, D), ml_dtypes.bfloat16)
        for g in range(G6):
            wout_p[g] = bf16r(W_out[chs[g], :])

        aperm = np.empty((P112, G6 * N), np.float32)
        bdtp = np.empty((P112, G6), np.float32)
        for g in range(G6):
            aperm[:, g * N:(g + 1) * N] = A[chs[g], :]
            bdtp[:, g] = b_dt[chs[g]]
        vth_p = v_th_d[chs[0]].astype(np.float32).reshape(P112, 1)

        in_maps.append({
            "hT": hdma, "wcat": wdma, "w18": w18_h, "csmat": csmat_h,
            "csb9": csb9_h, "selmu": selmu_h, "sel9": sel9_h,
            "wout": np.ascontiguousarray(wout_p),
            "aperm": aperm, "vb4": -SPIKE_BETA * vth_p, "vbn": -vth_p,
            "bdt": bdtp, "onesc": onesc_h, "ones1": ones1_h,
            "ones1f": np.ones((1, 128), np.float32),
        })
    return in_maps


def kernel(trace=False, **inputs):
    if "nc" not in _CACHE:
        _CACHE["nc"] = _build()
    nc = _CACHE["nc"]
    in_maps = _host_prep(inputs)
    res = run_bass_kernel_spmd(nc, in_maps, core_ids=list(range(NCORE)), trace=trace)
    out = np.empty((B, T, D), np.float32)
    for c in range(NCORE):
        b, q4 = c // 4, c % 4
        o = np.asarray(res.results[c]["out"], dtype=np.float32)  # (4, 128, D)
        for hh in range(4):
            out[b, hh * (T // 4) + q4 * 128: hh * (T // 4) + (q4 + 1) * 128, :] = o[hh]
    if trace:
        kernel.last_exec_time_ns = res.exec_time_ns
    return out
